# revision 1
# baseline (speedup 1.0000x reference)
"""CapsuleLayer (dynamic routing, 3 iterations) on 8 Trainium2 NeuronCores.

Decomposition (never materializes u_hat = [256,1152,10,16], 189MB):
  - Shard the 1152 input capsules (i) 8 ways: 144 per core.
  - Per-core row space j = (i_local, k), k = in_size = 8 -> 1152 rows
    = 9 chunks of 128 partitions.
  - s_j:  s[b,(n,o)] = sum_j xT[j,b] * (c[j,n] * Wl[j,(n,o)])   (PE matmul,
    contraction over j; Wl = 0.03*W in [(i,k),(n,o)] layout, c broadcast
    over k and o).  Partial over the i-shard -> AllReduce [256,160].
  - b_ij update via a Gram matrix instead of u_hat:
       Q[j,(n,o)]  = sum_b x[b,j] * v[b,(n,o)]                  (PE matmul)
       pr[j,n]     = sum_o Wl[j,(n,o)] * Q[j,(n,o)]             (DVE)
       uv_rows     = F.T @ pr  per 128-chunk, F = kron(I16, ones8x8)/B
                     (sums over k within each i-group AND replicates the
                     result back to all k-rows, so b stays row-replicated)
  - Iteration 1 uses uniform c = 1/10 (softmax of zeros): s1 = 0.1*(xT.T@Wl).
  - Iteration 3 needs no b-update; final s3 goes through ReduceScatter so
    each core squashes only its 32-row batch shard; host concatenates.
  - sqrt via exp(0.5*ln(x)) so Exp+Ln share one ACT table set (Sqrt lives
    in a different set and would force ~2.7us table reloads per iteration).
"""
import sys

if "/opt/trn_rl_repo" not in sys.path:
    sys.path.insert(0, "/opt/trn_rl_repo")

import numpy as np

N_CORES = 8
B, IN_SIZE, I_TOT = 256, 8, 1152
N_NODE, O_SZ = 10, 16
NO = N_NODE * O_SZ          # 160
I_SH = I_TOT // N_CORES     # 144 capsules per core
JR = I_SH * IN_SIZE         # 1152 rows per core
NCH = JR // 128             # 9 contraction chunks
BC = B // 128               # 2 batch chunks
B_SH = B // N_CORES         # 32 batch rows per core after ReduceScatter

_CACHE = {}


def _build_program():
    import concourse.bacc as bacc
    import concourse.tile as tile
    import concourse.mybir as mybir

    f32 = mybir.dt.float32
    AF = mybir.ActivationFunctionType
    ALU = mybir.AluOpType
    AX = mybir.AxisListType

    nc = bacc.Bacc("TRN2", target_bir_lowering=False, debug=False,
                   enable_asserts=True, num_devices=N_CORES)

    xt_d = nc.dram_tensor("xt", [JR, B], f32, kind="ExternalInput").ap()
    xik_d = nc.dram_tensor("xik", [B, JR], f32, kind="ExternalInput").ap()
    wl_d = nc.dram_tensor("wl", [JR, NO], f32, kind="ExternalInput").ap()
    f_d = nc.dram_tensor("fmat", [128, 128], f32, kind="ExternalInput").ap()
    y_d = nc.dram_tensor("y", [B_SH, NO], f32, kind="ExternalOutput").ap()

    RG = [list(range(N_CORES))]

    with tile.TileContext(nc) as tc:
        with tc.tile_pool(name="persist", bufs=1) as pp, \
             tc.tile_pool(name="work", bufs=1) as wp, \
             tc.tile_pool(name="ps_s", bufs=2, space="PSUM") as ps_s, \
             tc.tile_pool(name="ps_q", bufs=4, space="PSUM") as ps_q, \
             tc.tile_pool(name="ps_f", bufs=1, space="PSUM") as ps_f, \
             tc.tile_pool(name="dram", bufs=1, space="DRAM") as dp:

            # ---------------- input loads ----------------
            xt_sb = pp.tile([128, NCH, B], f32, name="xt_sb", tag="xt_sb")
            xik_sb = pp.tile([128, BC, JR], f32, name="xik_sb", tag="xik_sb")
            wl_sb = pp.tile([128, NCH, NO], f32, name="wl_sb", tag="wl_sb")
            f_sb = pp.tile([128, 128], f32, name="f_sb", tag="f_sb")
            b_sb = pp.tile([128, NCH, N_NODE], f32, name="b_sb", tag="b_sb")

            for c in range(NCH):
                nc.sync.dma_start(xt_sb[:, c, :], xt_d[c * 128:(c + 1) * 128, :])
                nc.sync.dma_start(wl_sb[:, c, :], wl_d[c * 128:(c + 1) * 128, :])
            for bc_i in range(BC):
                nc.sync.dma_start(xik_sb[:, bc_i, :],
                                  xik_d[bc_i * 128:(bc_i + 1) * 128, :])
            nc.sync.dma_start(f_sb[:], f_d[:])

            wl4 = wl_sb[:].rearrange("p c (n o) -> p c n o", n=N_NODE)

            # ---------------- helpers ----------------
            def s_matmul(rhs3, s_sb, scale):
                """s_sb[:,bc,:] = scale * sum_c xt[:,c,bc].T @ rhs3[:,c,:]"""
                for bc_i in range(BC):
                    s_ps = ps_s.tile([128, NO], f32, name="s_ps", tag="s_ps")
                    for c in range(NCH):
                        nc.tensor.matmul(
                            s_ps[:],
                            xt_sb[:, c, bc_i * 128:(bc_i + 1) * 128],
                            rhs3[:, c, :],
                            start=(c == 0), stop=(c == NCH - 1))
                    if scale is None:
                        nc.scalar.copy(s_sb[:, bc_i, :], s_ps[:])
                    else:
                        nc.scalar.mul(s_sb[:, bc_i, :], s_ps[:], scale)

            def squash(s_sb, P, nch, tag):
                """v = squash(s) over o. s_sb/v tiles [P, nch, NO]."""
                s4 = s_sb[:].rearrange("p c (n o) -> p c n o", n=N_NODE)
                sq = wp.tile([P, nch, NO], f32, name="sq" + tag, tag="sq" + tag)
                nc.vector.tensor_mul(sq[:], s_sb[:], s_sb[:])
                msq = wp.tile([P, nch, N_NODE], f32, name="msq" + tag,
                              tag="msq" + tag)
                nc.vector.reduce_sum(
                    msq[:], sq[:].rearrange("p c (n o) -> p c n o", n=N_NODE),
                    axis=AX.X)
                lnm = wp.tile([P, nch, N_NODE], f32, name="lnm" + tag,
                              tag="lnm" + tag)
                nc.scalar.activation(lnm[:], msq[:], AF.Ln)
                mag = wp.tile([P, nch, N_NODE], f32, name="mag" + tag,
                              tag="mag" + tag)
                nc.scalar.activation(mag[:], lnm[:], AF.Exp, scale=0.5)
                den = wp.tile([P, nch, N_NODE], f32, name="den" + tag,
                              tag="den" + tag)
                nc.vector.tensor_scalar_add(den[:], msq[:], 1.0)
                rden = wp.tile([P, nch, N_NODE], f32, name="rden" + tag,
                               tag="rden" + tag)
                nc.vector.reciprocal(rden[:], den[:])
                fac = wp.tile([P, nch, N_NODE], f32, name="fac" + tag,
                              tag="fac" + tag)
                nc.vector.tensor_mul(fac[:], mag[:], rden[:])
                v_sb = wp.tile([P, nch, NO], f32, name="v" + tag, tag="v" + tag)
                fb = fac[:].unsqueeze(3).broadcast_to((P, nch, N_NODE, O_SZ))
                nc.vector.tensor_mul(
                    v_sb[:].rearrange("p c (n o) -> p c n o", n=N_NODE), s4, fb)
                return v_sb

            def allreduce_s(s_sb, t):
                ar_in = dp.tile([B, NO], f32, name=f"ar_in{t}", tag="ar_in")
                ar_out = dp.tile([B, NO], f32, name=f"ar_out{t}", tag="ar_out")
                for bc_i in range(BC):
                    nc.sync.dma_start(ar_in[bc_i * 128:(bc_i + 1) * 128, :],
                                      s_sb[:, bc_i, :])
                nc.gpsimd.collective_compute(
                    "AllReduce", ALU.add, replica_groups=RG,
                    ins=[ar_in.opt()], outs=[ar_out.opt()])
                sfull = wp.tile([128, BC, NO], f32, name="sfull", tag="sfull")
                for bc_i in range(BC):
                    nc.sync.dma_start(sfull[:, bc_i, :],
                                      ar_out[bc_i * 128:(bc_i + 1) * 128, :])
                return sfull

            def b_update(v_sb, first):
                q_sb = wp.tile([128, NCH, NO], f32, name="q_sb", tag="q_sb")
                for mc in range(NCH):
                    q_ps = ps_q.tile([128, NO], f32, name="q_ps", tag="q_ps")
                    for bc_i in range(BC):
                        nc.tensor.matmul(
                            q_ps[:],
                            xik_sb[:, bc_i, mc * 128:(mc + 1) * 128],
                            v_sb[:, bc_i, :],
                            start=(bc_i == 0), stop=(bc_i == BC - 1))
                    nc.scalar.copy(q_sb[:, mc, :], q_ps[:])
                p_sb = wp.tile([128, NCH, NO], f32, name="p_sb", tag="p_sb")
                nc.vector.tensor_mul(p_sb[:], wl_sb[:], q_sb[:])
                pr = wp.tile([128, NCH, N_NODE], f32, name="pr_sb", tag="pr_sb")
                nc.vector.reduce_sum(
                    pr[:], p_sb[:].rearrange("p c (n o) -> p c n o", n=N_NODE),
                    axis=AX.X)
                uv_ps = ps_f.tile([128, NCH * N_NODE], f32, name="uv_ps",
                                  tag="uv_ps")
                nc.tensor.matmul(uv_ps[:], f_sb[:],
                                 pr[:].rearrange("p c n -> p (c n)"),
                                 start=True, stop=True)
                uv3 = uv_ps[:].rearrange("p (c n) -> p c n", n=N_NODE)
                if first:
                    nc.scalar.copy(b_sb[:], uv3)
                else:
                    nc.vector.tensor_add(b_sb[:], b_sb[:], uv3)

            def softmax_mc():
                e_sb = wp.tile([128, NCH, N_NODE], f32, name="e_sb", tag="e_sb")
                nc.scalar.activation(e_sb[:], b_sb[:], AF.Exp)
                se = wp.tile([128, NCH], f32, name="se", tag="se")
                nc.vector.reduce_sum(se[:], e_sb[:], axis=AX.X)
                rse = wp.tile([128, NCH], f32, name="rse", tag="rse")
                nc.vector.reciprocal(rse[:], se[:])
                c_sb = wp.tile([128, NCH, N_NODE], f32, name="c_sb", tag="c_sb")
                nc.vector.tensor_mul(
                    c_sb[:], e_sb[:],
                    rse[:].unsqueeze(2).broadcast_to((128, NCH, N_NODE)))
                mc_sb = wp.tile([128, NCH, NO], f32, name="mc_sb", tag="mc_sb")
                cb = c_sb[:].unsqueeze(3).broadcast_to(
                    (128, NCH, N_NODE, O_SZ))
                nc.vector.tensor_mul(
                    mc_sb[:].rearrange("p c (n o) -> p c n o", n=N_NODE),
                    wl4, cb)
                return mc_sb

            # ---------------- iteration 1 (c uniform = 0.1) ----------------
            s_sb = wp.tile([128, BC, NO], f32, name="s_sb", tag="s_sb")
            s_matmul(wl_sb[:], s_sb, scale=0.1)
            sfull = allreduce_s(s_sb, 0)
            v_sb = squash(sfull, 128, BC, "m")
            b_update(v_sb, first=True)

            # ---------------- iteration 2 ----------------
            mc_sb = softmax_mc()
            s_sb = wp.tile([128, BC, NO], f32, name="s_sb", tag="s_sb")
            s_matmul(mc_sb[:], s_sb, scale=None)
            sfull = allreduce_s(s_sb, 1)
            v_sb = squash(sfull, 128, BC, "m")
            b_update(v_sb, first=False)

            # ---------------- iteration 3 (no b-update) ----------------
            mc_sb = softmax_mc()
            s_sb = wp.tile([128, BC, NO], f32, name="s_sb", tag="s_sb")
            s_matmul(mc_sb[:], s_sb, scale=None)

            rs_in = dp.tile([B, NO], f32, name="rs_in", tag="rs_in")
            rs_out = dp.tile([B_SH, NO], f32, name="rs_out", tag="rs_out")
            for bc_i in range(BC):
                nc.sync.dma_start(rs_in[bc_i * 128:(bc_i + 1) * 128, :],
                                  s_sb[:, bc_i, :])
            nc.gpsimd.collective_compute(
                "ReduceScatter", ALU.add, replica_groups=RG,
                ins=[rs_in.opt()], outs=[rs_out.opt()])
            ssh = wp.tile([B_SH, 1, NO], f32, name="ssh", tag="ssh")
            nc.sync.dma_start(ssh[:, 0, :], rs_out[:])
            vsh = squash(ssh, B_SH, 1, "s")
            nc.sync.dma_start(y_d[:], vsh[:, 0, :])

    nc.compile()
    return nc


def _host_prep(x, W):
    """Per-core input dicts + the constant F matrix."""
    x = np.ascontiguousarray(x, dtype=np.float32)
    W = np.ascontiguousarray(W, dtype=np.float32)
    F = (np.kron(np.eye(16, dtype=np.float32),
                 np.ones((8, 8), dtype=np.float32)) / np.float32(B))
    in_maps = []
    for c in range(N_CORES):
        sl = slice(c * I_SH, (c + 1) * I_SH)
        x_sh = x[:, :, sl]                                   # [B, K, I_SH]
        xt = np.ascontiguousarray(x_sh.transpose(2, 1, 0)).reshape(JR, B)
        xik = np.ascontiguousarray(x_sh.transpose(0, 2, 1)).reshape(B, JR)
        wl = np.ascontiguousarray(
            (np.float32(0.03) * W[0, sl]).transpose(0, 3, 1, 2)).reshape(JR, NO)
        in_maps.append({"xt": xt, "xik": xik, "wl": wl, "fmat": F})
    return in_maps


def _run(in_maps, trace=False):
    from concourse.bass_utils import run_bass_kernel_spmd

    if "nc" not in _CACHE:
        _CACHE["nc"] = _build_program()
    nc = _CACHE["nc"]
    res = run_bass_kernel_spmd(nc, in_maps, core_ids=list(range(N_CORES)),
                               trace=trace)
    return res


def kernel(x: np.ndarray, W: np.ndarray) -> np.ndarray:
    in_maps = _host_prep(x, W)
    res = _run(in_maps)
    v = np.concatenate([res.results[c]["y"] for c in range(N_CORES)], axis=0)
    return v.reshape(B, N_NODE, O_SZ, 1).astype(np.float32)


# revision 8
# speedup vs baseline: 1.0628x; 1.0628x over previous
"""CapsuleLayer (dynamic routing, 3 iterations) on 8 Trainium2 NeuronCores.

Decomposition (never materializes u_hat = [256,1152,10,16], 189MB):
  - Shard the 1152 input capsules (i) 8 ways: 144 per core.
  - Per-core row space j = (i_local, k), k = in_size = 8 -> 1152 rows
    = 9 chunks of 128 partitions.
  - s_j:  s[b,(n,o)] = sum_j xT[j,b] * (c[j,n] * Wl[j,(n,o)])   (PE matmul,
    contraction over j; Wl = 0.03*W in [(i,k),(n,o)] layout, c broadcast
    over k and o).  Partial over the i-shard -> AllReduce [256,160].
  - b_ij update via a Gram matrix instead of u_hat:
       Q[j,(n,o)]  = sum_b x[b,j] * v[b,(n,o)]                  (PE matmul)
       pr[j,n]     = sum_o Wl[j,(n,o)] * Q[j,(n,o)]             (DVE)
       uv_rows     = F.T @ pr  per 128-chunk, F = kron(I16, ones8x8)/B
                     (sums over k within each i-group AND replicates the
                     result back to all k-rows, so b stays row-replicated)
  - Iteration 1 uses uniform c = 1/10 (softmax of zeros): s1 = 0.1*(xT.T@Wl).
  - Iteration 3 needs no b-update; final s3 goes through ReduceScatter so
    each core squashes only its 32-row batch shard; host concatenates.

Precision plan: matmul operands are bf16 (PSUM accumulates fp32; fp32
matmuls on trn2 lower to 2x LDWEIGHTS + 2x dual-pass MATMUL, ~8x slower).
The AllReduce / squash / softmax state stays fp32.  sqrt is done with a
bit-trick + 2 Newton steps on the DVE so the ScalarE only ever needs one
activation-table set (Exp); Sqrt/Ln live in other sets and would force
~2.7us ACT_TABLE_LOADs per use.
"""
import sys

if "/opt/trn_rl_repo" not in sys.path:
    sys.path.insert(0, "/opt/trn_rl_repo")

import numpy as np

N_CORES = 8
B, IN_SIZE, I_TOT = 256, 8, 1152
N_NODE, O_SZ = 10, 16
NO = N_NODE * O_SZ          # 160
I_SH = I_TOT // N_CORES     # 144 capsules per core
JR = I_SH * IN_SIZE         # 1152 rows per core
NCH = JR // 128             # 9 contraction chunks
BC = B // 128               # 2 batch chunks
B_SH = B // N_CORES         # 32 batch rows per core after ReduceScatter

RSQRT_MAGIC = 0x5F3759DF

_CACHE = {}


def _build_program():
    import concourse.bacc as bacc
    import concourse.tile as tile
    import concourse.mybir as mybir

    f32 = mybir.dt.float32
    bf16 = mybir.dt.bfloat16
    i32 = mybir.dt.int32
    AF = mybir.ActivationFunctionType
    ALU = mybir.AluOpType
    AX = mybir.AxisListType

    nc = bacc.Bacc("TRN2", target_bir_lowering=False, debug=False,
                   enable_asserts=True, num_devices=N_CORES)

    xt_d = nc.dram_tensor("xt", [JR, B], bf16, kind="ExternalInput").ap()
    xtl_d = nc.dram_tensor("xtl", [JR, B], bf16, kind="ExternalInput").ap()
    xik_d = nc.dram_tensor("xik", [B, JR], bf16, kind="ExternalInput").ap()
    wl_d = nc.dram_tensor("wl", [JR, NO], bf16, kind="ExternalInput").ap()
    wlf_d = nc.dram_tensor("wlf", [JR, NO], f32, kind="ExternalInput").ap()
    f_d = nc.dram_tensor("fmat", [128, 128], bf16, kind="ExternalInput").ap()
    y_d = nc.dram_tensor("y", [B_SH, NO], f32, kind="ExternalOutput").ap()

    RG = [list(range(N_CORES))]

    with tile.TileContext(nc) as tc:
        with tc.tile_pool(name="persist", bufs=1) as pp, \
             tc.tile_pool(name="work", bufs=1) as wp, \
             tc.tile_pool(name="ps_s", bufs=2, space="PSUM") as ps_s, \
             tc.tile_pool(name="ps_q", bufs=3, space="PSUM") as ps_q, \
             tc.tile_pool(name="ps_f", bufs=1, space="PSUM") as ps_f, \
             tc.tile_pool(name="dram", bufs=1, space="DRAM") as dp:

            # ---------------- input loads ----------------
            xt_sb = pp.tile([128, NCH, B], bf16, name="xt_sb", tag="xt_sb")
            xtl_sb = pp.tile([128, NCH, B], bf16, name="xtl_sb", tag="xtl_sb")
            xik_sb = pp.tile([128, BC, JR], bf16, name="xik_sb", tag="xik_sb")
            wl_sb = pp.tile([128, NCH, NO], bf16, name="wl_sb", tag="wl_sb")
            wlf_sb = pp.tile([128, NCH, NO], f32, name="wlf_sb", tag="wlf_sb")
            f_sb = pp.tile([128, 128], bf16, name="f_sb", tag="f_sb")
            b_sb = pp.tile([128, NCH, N_NODE], f32, name="b_sb", tag="b_sb")

            for c in range(NCH):
                nc.sync.dma_start(xt_sb[:, c, :], xt_d[c * 128:(c + 1) * 128, :])
                nc.sync.dma_start(xtl_sb[:, c, :],
                                  xtl_d[c * 128:(c + 1) * 128, :])
                nc.sync.dma_start(wl_sb[:, c, :], wl_d[c * 128:(c + 1) * 128, :])
                nc.sync.dma_start(wlf_sb[:, c, :],
                                  wlf_d[c * 128:(c + 1) * 128, :])
            for bc_i in range(BC):
                nc.sync.dma_start(xik_sb[:, bc_i, :],
                                  xik_d[bc_i * 128:(bc_i + 1) * 128, :])
            nc.sync.dma_start(f_sb[:], f_d[:])

            wl4 = wl_sb[:].rearrange("p c (n o) -> p c n o", n=N_NODE)

            # ---------------- helpers ----------------
            def s_matmul(rhs3, s_sb, scale):
                """s_sb[:,bc,:] = scale * sum_c xt[:,c,bc].T @ rhs3[:,c,:]"""
                for bc_i in range(BC):
                    s_ps = ps_s.tile([128, NO], f32, name="s_ps", tag="s_ps")
                    for c in range(NCH):
                        nc.tensor.matmul(
                            s_ps[:],
                            xt_sb[:, c, bc_i * 128:(bc_i + 1) * 128],
                            rhs3[:, c, :],
                            start=(c == 0), stop=(c == NCH - 1))
                    if scale is None:
                        nc.scalar.copy(s_sb[:, bc_i, :], s_ps[:])
                    else:
                        nc.scalar.mul(s_sb[:, bc_i, :], s_ps[:], scale)

            def rsqrt(msq, P, nch, tag):
                """z ~ 1/sqrt(msq) via int bit-trick + 2 Newton steps (DVE
                only -- avoids the Sqrt/Ln ACT table sets entirely)."""
                sh = [P, nch, N_NODE]
                zi = wp.tile(sh, i32, name="zi" + tag, tag="zi" + tag)
                # zi = ((bits >> 1) ^ -1) + (MAGIC + 1)  ==  MAGIC - (bits>>1)
                nc.vector.tensor_scalar(
                    out=zi[:], in0=msq[:].bitcast(i32), scalar1=1, scalar2=-1,
                    op0=ALU.arith_shift_right, op1=ALU.bitwise_xor)
                nc.vector.tensor_scalar_add(zi[:], zi[:], RSQRT_MAGIC + 1)
                z = zi[:].bitcast(f32)
                t = wp.tile(sh, f32, name="nt" + tag, tag="nt" + tag)
                w = wp.tile(sh, f32, name="nw" + tag, tag="nw" + tag)
                for _ in range(2):
                    nc.vector.tensor_mul(t[:], z, z)
                    nc.vector.tensor_mul(t[:], t[:], msq[:])
                    nc.vector.tensor_scalar(
                        out=w[:], in0=t[:], scalar1=-0.5, scalar2=1.5,
                        op0=ALU.mult, op1=ALU.add)
                    nc.vector.tensor_mul(z, z, w[:])
                return z

            def squash(s_sb, P, nch, tag, v_dtype):
                """v = squash(s) over o. s_sb [P, nch, NO] fp32."""
                s4 = s_sb[:].rearrange("p c (n o) -> p c n o", n=N_NODE)
                sq = wp.tile([P, nch, NO], f32, name="sq" + tag, tag="sq" + tag)
                nc.vector.tensor_mul(sq[:], s_sb[:], s_sb[:])
                msq = wp.tile([P, nch, N_NODE], f32, name="msq" + tag,
                              tag="msq" + tag)
                nc.vector.reduce_sum(
                    msq[:], sq[:].rearrange("p c (n o) -> p c n o", n=N_NODE),
                    axis=AX.X)
                z = rsqrt(msq, P, nch, tag)           # 1/sqrt(msq)
                mag = wp.tile([P, nch, N_NODE], f32, name="mag" + tag,
                              tag="mag" + tag)
                nc.vector.tensor_mul(mag[:], msq[:], z)   # sqrt(msq)
                den = wp.tile([P, nch, N_NODE], f32, name="den" + tag,
                              tag="den" + tag)
                nc.vector.tensor_scalar_add(den[:], msq[:], 1.0)
                rden = wp.tile([P, nch, N_NODE], f32, name="rden" + tag,
                               tag="rden" + tag)
                nc.vector.reciprocal(rden[:], den[:])
                fac = wp.tile([P, nch, N_NODE], f32, name="fac" + tag,
                              tag="fac" + tag)
                nc.vector.tensor_mul(fac[:], mag[:], rden[:])
                v_sb = wp.tile([P, nch, NO], v_dtype, name="v" + tag,
                               tag="v" + tag)
                fb = fac[:].unsqueeze(3).broadcast_to((P, nch, N_NODE, O_SZ))
                nc.vector.tensor_mul(
                    v_sb[:].rearrange("p c (n o) -> p c n o", n=N_NODE), s4, fb)
                return v_sb

            def allreduce_s(s_sb, t):
                ar_in = dp.tile([B, NO], f32, name=f"ar_in{t}", tag="ar_in")
                ar_out = dp.tile([B, NO], f32, name=f"ar_out{t}", tag="ar_out")
                for bc_i in range(BC):
                    nc.sync.dma_start(ar_in[bc_i * 128:(bc_i + 1) * 128, :],
                                      s_sb[:, bc_i, :])
                nc.gpsimd.collective_compute(
                    "AllReduce", ALU.add, replica_groups=RG,
                    ins=[ar_in.opt()], outs=[ar_out.opt()])
                sfull = wp.tile([128, BC, NO], f32, name="sfull", tag="sfull")
                for bc_i in range(BC):
                    nc.sync.dma_start(sfull[:, bc_i, :],
                                      ar_out[bc_i * 128:(bc_i + 1) * 128, :])
                return sfull

            def b_update(v_sb, first):
                q_sb = wp.tile([128, NCH, NO], bf16, name="q_sb", tag="q_sb")
                for mc in range(NCH):
                    q_ps = ps_q.tile([128, NO], f32, name="q_ps", tag="q_ps")
                    for bc_i in range(BC):
                        nc.tensor.matmul(
                            q_ps[:],
                            xik_sb[:, bc_i, mc * 128:(mc + 1) * 128],
                            v_sb[:, bc_i, :],
                            start=(bc_i == 0), stop=(bc_i == BC - 1))
                    nc.scalar.copy(q_sb[:, mc, :], q_ps[:])
                p_sb = wp.tile([128, NCH, NO], bf16, name="p_sb", tag="p_sb")
                nc.vector.tensor_mul(p_sb[:], wl_sb[:], q_sb[:])
                pr = wp.tile([128, NCH, N_NODE], f32, name="pr_sb", tag="pr_sb")
                nc.vector.reduce_sum(
                    pr[:], p_sb[:].rearrange("p c (n o) -> p c n o", n=N_NODE),
                    axis=AX.X)
                prb = wp.tile([128, NCH, N_NODE], bf16, name="prb", tag="prb")
                nc.scalar.copy(prb[:], pr[:])
                uv_ps = ps_f.tile([128, NCH * N_NODE], f32, name="uv_ps",
                                  tag="uv_ps")
                nc.tensor.matmul(uv_ps[:], f_sb[:],
                                 prb[:].rearrange("p c n -> p (c n)"),
                                 start=True, stop=True)
                uv3 = uv_ps[:].rearrange("p (c n) -> p c n", n=N_NODE)
                if first:
                    nc.scalar.copy(b_sb[:], uv3)
                else:
                    nc.vector.tensor_add(b_sb[:], b_sb[:], uv3)

            def softmax_c(c_dtype):
                e_sb = wp.tile([128, NCH, N_NODE], f32, name="e_sb", tag="e_sb")
                nc.scalar.activation(e_sb[:], b_sb[:], AF.Exp)
                se = wp.tile([128, NCH], f32, name="se", tag="se")
                nc.vector.reduce_sum(se[:], e_sb[:], axis=AX.X)
                rse = wp.tile([128, NCH], f32, name="rse", tag="rse")
                nc.vector.reciprocal(rse[:], se[:])
                c_sb = wp.tile([128, NCH, N_NODE], c_dtype, name="c_sb",
                               tag="c_sb" + str(c_dtype))
                nc.vector.tensor_mul(
                    c_sb[:], e_sb[:],
                    rse[:].unsqueeze(2).broadcast_to((128, NCH, N_NODE)))
                return c_sb

            def softmax_mc():
                c_sb = softmax_c(bf16)
                mc_sb = wp.tile([128, NCH, NO], bf16, name="mc_sb", tag="mc_sb")
                cb = c_sb[:].unsqueeze(3).broadcast_to(
                    (128, NCH, N_NODE, O_SZ))
                nc.vector.tensor_mul(
                    mc_sb[:].rearrange("p c (n o) -> p c n o", n=N_NODE),
                    wl4, cb)
                return mc_sb

            # ---------------- iteration 1 (c uniform = 0.1) ----------------
            s_sb = wp.tile([128, BC, NO], f32, name="s_sb", tag="s_sb")
            s_matmul(wl_sb[:], s_sb, scale=0.1)
            sfull = allreduce_s(s_sb, 0)
            v_sb = squash(sfull, 128, BC, "m", bf16)
            b_update(v_sb, first=True)

            # ---------------- iteration 2 ----------------
            mc_sb = softmax_mc()
            s_sb = wp.tile([128, BC, NO], f32, name="s_sb", tag="s_sb")
            s_matmul(mc_sb[:], s_sb, scale=None)
            sfull = allreduce_s(s_sb, 1)
            v_sb = squash(sfull, 128, BC, "m", bf16)
            b_update(v_sb, first=False)

            # ---------------- iteration 3 (no b-update) ----------------
            # Output-determining pass: fp32 c3/mc3, then a 3-product Dekker
            # split so the bf16 PE reproduces the fp32 matmul to ~1e-5:
            #   s3 = xtH.T @ mcH  +  xtH.T @ mcL  +  xtL.T @ mcH
            c3 = softmax_c(f32)
            mc3 = wp.tile([128, NCH, NO], f32, name="mc3", tag="mc3")
            cb3 = c3[:].unsqueeze(3).broadcast_to((128, NCH, N_NODE, O_SZ))
            wlf4 = wlf_sb[:].rearrange("p c (n o) -> p c n o", n=N_NODE)
            nc.vector.tensor_mul(
                mc3[:].rearrange("p c (n o) -> p c n o", n=N_NODE), wlf4, cb3)
            # hi/lo split packed pairwise so one matmul streams [mcH | mcL]
            mcp = wp.tile([128, NCH, 2, NO], bf16, name="mcp", tag="mcp")
            nc.scalar.copy(mcp[:, :, 0, :], mc3[:])
            nc.vector.tensor_sub(mcp[:, :, 1, :], mc3[:], mcp[:, :, 0, :])

            s_sb = wp.tile([128, BC, NO], f32, name="s_sb", tag="s_sb")
            for bc_i in range(BC):
                ps_a = ps_s.tile([128, 2 * NO], f32, name="ps_a", tag="ps_a")
                ps_c = ps_s.tile([128, NO], f32, name="s_ps", tag="s_ps")
                for c in range(NCH):
                    lhs_h = xt_sb[:, c, bc_i * 128:(bc_i + 1) * 128]
                    lhs_l = xtl_sb[:, c, bc_i * 128:(bc_i + 1) * 128]
                    nc.tensor.matmul(
                        ps_a[:], lhs_h,
                        mcp[:, c, :, :].rearrange("p t f -> p (t f)"),
                        start=(c == 0), stop=(c == NCH - 1))
                    nc.tensor.matmul(
                        ps_c[:], lhs_l, mcp[:, c, 0, :],
                        start=(c == 0), stop=(c == NCH - 1))
                nc.scalar.copy(s_sb[:, bc_i, :], ps_a[:, 0:NO])
                nc.vector.tensor_add(s_sb[:, bc_i, :], s_sb[:, bc_i, :],
                                     ps_a[:, NO:2 * NO])
                nc.vector.tensor_add(s_sb[:, bc_i, :], s_sb[:, bc_i, :],
                                     ps_c[:])

            rs_in = dp.tile([B, NO], f32, name="rs_in", tag="rs_in")
            rs_out = dp.tile([B_SH, NO], f32, name="rs_out", tag="rs_out")
            for bc_i in range(BC):
                nc.sync.dma_start(rs_in[bc_i * 128:(bc_i + 1) * 128, :],
                                  s_sb[:, bc_i, :])
            nc.gpsimd.collective_compute(
                "ReduceScatter", ALU.add, replica_groups=RG,
                ins=[rs_in.opt()], outs=[rs_out.opt()])
            ssh = wp.tile([B_SH, 1, NO], f32, name="ssh", tag="ssh")
            nc.sync.dma_start(ssh[:, 0, :], rs_out[:])
            vsh = squash(ssh, B_SH, 1, "s", f32)
            nc.sync.dma_start(y_d[:], vsh[:, 0, :])

    nc.compile()
    return nc


def _host_prep(x, W):
    """Per-core input dicts + the constant F matrix."""
    import ml_dtypes

    bf = ml_dtypes.bfloat16
    x = np.ascontiguousarray(x, dtype=np.float32)
    W = np.ascontiguousarray(W, dtype=np.float32)
    F = (np.kron(np.eye(16, dtype=np.float32),
                 np.ones((8, 8), dtype=np.float32)) / np.float32(B)).astype(bf)
    in_maps = []
    for c in range(N_CORES):
        sl = slice(c * I_SH, (c + 1) * I_SH)
        x_sh = x[:, :, sl]                                   # [B, K, I_SH]
        xt = np.ascontiguousarray(x_sh.transpose(2, 1, 0)).reshape(JR, B)
        xt_hi = xt.astype(bf)
        xt_lo = (xt - xt_hi.astype(np.float32)).astype(bf)
        xik = np.ascontiguousarray(
            x_sh.transpose(0, 2, 1)).reshape(B, JR).astype(bf)
        wlf = np.ascontiguousarray(
            (np.float32(0.03) * W[0, sl]).transpose(0, 3, 1, 2)
        ).reshape(JR, NO)
        in_maps.append({"xt": xt_hi, "xtl": xt_lo, "xik": xik,
                        "wl": wlf.astype(bf), "wlf": wlf, "fmat": F})
    return in_maps


def _run(in_maps, trace=False):
    from concourse.bass_utils import run_bass_kernel_spmd

    if "nc" not in _CACHE:
        _CACHE["nc"] = _build_program()
    nc = _CACHE["nc"]
    res = run_bass_kernel_spmd(nc, in_maps, core_ids=list(range(N_CORES)),
                               trace=trace)
    return res


def kernel(x: np.ndarray, W: np.ndarray) -> np.ndarray:
    in_maps = _host_prep(x, W)
    res = _run(in_maps)
    v = np.concatenate([res.results[c]["y"] for c in range(N_CORES)], axis=0)
    return v.reshape(B, N_NODE, O_SZ, 1).astype(np.float32)
